# revision 1
# baseline (speedup 1.0000x reference)
"""Conv2d(128->256, 3x3, stride 1, pad 1) on (32,128,56,56) fp32, data-parallel
over 8 NeuronCores, computed in fp8e4 (e4m3) with DoubleRow matmuls.

Per core (4 images):
  - Host splits x and w into fp8 hi + lo parts: xh = fp8(x), xl = fp8(x - xh),
    wh = fp8(w), wl = fp8(w - wh). The conv is computed as
        (xh + xl) * wh  (all 9 taps)  +  xh * wl  (taps 0..5)
    giving ~1.44e-2 rel fro error incl. the bf16 output round (gate 2e-2).
  - DoubleRow perf mode contracts 2 k-tiles (2x128 K values) per instruction
    at 0.5 cycles/row -- 2x the bf16/f32r rate.  K-tile pairs are built as
    overlapping strided SBUF views (hand-written access patterns):
      * (xh, xl) hi/lo pairs for the wh terms: k-tile stride = hi->lo offset
      * (tap t, tap t+3) pairs for the wl terms: k-tile stride = 58 (one
        padded row).  NB a k-tile stride of 1 hard-crashes the PE when the
        matmul is not first in its accumulation group, so taps pair
        vertically, never horizontally.
  - 12 DoubleRow matmuls per 8-row output chunk (N=448, one PSUM bank),
    7 chunks x 2 out-halves x 4 images = 672 matmuls x 93ns = 62.7us PE.
  - PSUM -> SBUF copy fuses the bias add (ScalarE/VectorE alternating) and
    narrows to bf16, halving the output DMA; the host widens back to fp32.
"""

import numpy as np
import ml_dtypes

import bass_rust
import concourse.bass as bass  # noqa: F401
import concourse.mybir as mybir
import concourse.tile as tile
from concourse import bacc
from concourse.bass_utils import run_bass_kernel_spmd

N_CORES = 8
N_IMG = 4  # images per core
C_IN = 128
C_OUT = 256
H = W = 56
HP = WP = 58
SP = HP * WP  # 3364 padded spatial
SO = H * W  # 3136 output spatial
NROW = 8  # output rows per PSUM chunk
NCH = NROW * W  # 448 columns per matmul
RCHUNKS = H // NROW  # 7
NTAP = 9
TAP_OFF = [kh * WP + kw for kh in range(3) for kw in range(3)]

F8 = mybir.dt.float8e4
NP8 = ml_dtypes.float8_e4m3

_CACHE = {}


def _sv(ap_obj, dims, extra=0):
    """Hand-built (possibly overlapping) strided view of an AP."""
    c = ap_obj.copy()
    c.ap = bass_rust.VecI64Pair([list(d) for d in dims])
    c.offset = c.offset + extra
    return c


def _build_module():
    nc = bacc.Bacc("TRN2", target_bir_lowering=False, debug=False)

    f32 = mybir.dt.float32
    bf16 = mybir.dt.bfloat16
    DR = mybir.MatmulPerfMode.DoubleRow

    # x8: [hi/lo, img, chan, padded-spatial] fp8
    x8 = nc.dram_tensor("x8", [2, N_IMG, C_IN, SP], F8, kind="ExternalInput").ap()
    # wts: [c, o2, slot24, 128] fp8: slots 2t,2t+1 = (wh[t], wh[t]) for the 9
    # hi/lo-pair matmuls; slots 18+2p,19+2p = (wl[p], wl[p+3]) for the 3
    # correction pairs.
    wts = nc.dram_tensor("wts", [C_IN, 2 * 24 * 128], F8, kind="ExternalInput").ap()
    br = nc.dram_tensor("br", [C_IN, 2], f32, kind="ExternalInput").ap()
    out = nc.dram_tensor("out", [N_IMG, C_OUT, SO], bf16, kind="ExternalOutput").ap()

    wts_v = wts.rearrange("c (h s o) -> c h s o", h=2, s=24)

    with tile.TileContext(nc) as tc:
        with (
            tc.tile_pool(name="const", bufs=1) as cpool,
            tc.tile_pool(name="osb", bufs=3) as opool,
            tc.tile_pool(name="pp", bufs=8, space="PSUM") as ppool,
        ):
            x_sb = cpool.tile([C_IN, 2, N_IMG, SP], F8)
            w_sb = cpool.tile([C_IN, 2, 24, 128], F8)
            b_sb = cpool.tile([C_IN, 2], f32)

            # ---- PE clock warmup: pin pe_busy_start as early as possible.
            # (the HAM p-state ramp counts from the first PE activity; a few
            # dummy f32 matmuls on a zeroed scratch tile suffice -- idle gaps
            # before the real stream do not reset the ramp.  The memzero goes
            # on Pool, which is free right after the entry barrier.)
            WARM_N = 64
            warm_sb = cpool.tile([C_IN, WARM_N], f32)
            nc.gpsimd.memzero(warm_sb)
            ps_warm = ppool.tile([128, NCH], f32, tag="ps")
            N_WARM = 8
            for i in range(N_WARM):
                nc.tensor.matmul(
                    ps_warm[:WARM_N, :WARM_N],
                    lhsT=warm_sb[:, :WARM_N],
                    rhs=warm_sb,
                    start=(i == 0),
                    stop=(i == N_WARM - 1),
                )
            # Keep the Pool DGE busy for ~3.5us so the gpsimd bulk transfers
            # below don't contend with img0's head-critical bands on the
            # shared DMA pipe.
            delay_sb = cpool.tile([C_IN, 3000], F8)
            nc.gpsimd.memzero(delay_sb)

            # ---- DMA plan: head-critical pieces first on the SP queue ----
            # chunk (n=0, o2=0, r) needs: wts half 0, x img0 hi+lo rows
            # <= 8r+9.  Stream img0 in row bands (hi+lo merged per band);
            # o2=1 weights and imgs 1-3 follow on the gpsimd queue.
            nc.sync.dma_start(out=w_sb[:, 0, 0:12], in_=wts_v[:, 0, 0:12])
            nc.sync.dma_start(out=x_sb[:, :, 0, : 10 * WP], in_=x8[:, 0, :, : 10 * WP].transpose([1, 0, 2]))
            nc.sync.dma_start(out=w_sb[:, 0, 12:24], in_=wts_v[:, 0, 12:24])
            nc.sync.dma_start(
                out=x_sb[:, :, 0, 10 * WP : 18 * WP],
                in_=x8[:, 0, :, 10 * WP : 18 * WP].transpose([1, 0, 2]),
            )
            nc.sync.dma_start(
                out=x_sb[:, :, 0, 18 * WP : 34 * WP],
                in_=x8[:, 0, :, 18 * WP : 34 * WP].transpose([1, 0, 2]),
            )
            nc.sync.dma_start(
                out=x_sb[:, :, 0, 34 * WP :], in_=x8[:, 0, :, 34 * WP :].transpose([1, 0, 2])
            )
            nc.gpsimd.dma_start(out=b_sb, in_=br)
            nc.gpsimd.dma_start(out=w_sb[:, 1], in_=wts_v[:, 1])
            for n in range(1, N_IMG):
                nc.gpsimd.dma_start(
                    out=x_sb[:, :, n, :], in_=x8[:, n, :, :].transpose([1, 0, 2])
                )

            # strides for the hand-built rhs views
            hi0 = x_sb[:, 0, 0, :]
            pstride = hi0.ap[0][0]
            d_lo = x_sb[:, 1, 0, :].offset - hi0.offset  # hi -> lo k-tile stride

            out_q = 0  # alternate output stores across both DMA queues
            for n in range(N_IMG):
                base = x_sb[:, 0, n, :]  # hi plane of image n
                for o2 in range(2):
                    o_sb = opool.tile([128, SO], bf16, tag="o_sb")
                    for r in range(RCHUNKS):
                        is_last = n == N_IMG - 1 and o2 == 1 and r == RCHUNKS - 1
                        bias_ap = b_sb[:, o2 : o2 + 1]
                        o_slice = out[n, o2 * 128 : (o2 + 1) * 128, r * NCH : (r + 1) * NCH]

                        def chunk_matmuls(ps, r0, col0, ncol):
                            # (xh + xl) * wh : all 9 taps, hi/lo k-tile pairs
                            for t in range(NTAP):
                                rhs = _sv(
                                    base,
                                    [[pstride, 128], [d_lo, 2], [WP, ncol // W], [1, W]],
                                    extra=r0 + col0 + TAP_OFF[t],
                                )
                                nc.tensor.matmul(
                                    ps,
                                    lhsT=w_sb[:, o2, 2 * t : 2 * t + 2, :],
                                    rhs=rhs,
                                    start=(t == 0),
                                    stop=False,
                                    perf_mode=DR,
                                )
                            # xh * wl : taps (p, p+3) pairs, k-tile stride 58
                            for p in range(3):
                                rhs = _sv(
                                    base,
                                    [
                                        [pstride, 128],
                                        [TAP_OFF[p + 3] - TAP_OFF[p], 2],
                                        [WP, ncol // W],
                                        [1, W],
                                    ],
                                    extra=r0 + col0 + TAP_OFF[p],
                                )
                                nc.tensor.matmul(
                                    ps,
                                    lhsT=w_sb[:, o2, 18 + 2 * p : 18 + 2 * p + 2, :],
                                    rhs=rhs,
                                    start=False,
                                    stop=(p == 2),
                                    perf_mode=DR,
                                )

                        r0 = r * NROW * WP
                        if is_last:
                            # tail chunk: two half-groups (N=224) so draining
                            # starts before the final matmul; copies on both
                            # engines into private tiles (no false deps) and
                            # stores spread over both DMA queues
                            hc = NCH // 2
                            qc = NCH // 4
                            for half in range(2):
                                psh = ppool.tile([128, hc], f32, tag="ps")
                                chunk_matmuls(psh, r0, half * hc // W * WP, hc)
                                for qq in range(2):
                                    q = 2 * half + qq
                                    s_ps = slice(qq * qc, (qq + 1) * qc)
                                    s_out = slice(q * qc, (q + 1) * qc)
                                    t_sb = opool.tile([128, qc], bf16, tag=f"tail{q}")
                                    if qq == 0:
                                        nc.vector.tensor_scalar_add(t_sb, psh[:, s_ps], bias_ap)
                                    else:
                                        nc.scalar.activation(
                                            t_sb,
                                            psh[:, s_ps],
                                            mybir.ActivationFunctionType.Identity,
                                            bias=bias_ap,
                                        )
                                    eng = nc.sync if qq == 0 else nc.gpsimd
                                    eng.dma_start(out=o_slice[:, s_out], in_=t_sb)
                        else:
                            ps = ppool.tile([128, NCH], f32, tag="ps")
                            chunk_matmuls(ps, r0, 0, NCH)
                            dst = o_sb[:, r * NCH : (r + 1) * NCH]
                            if r % 2 == 0:
                                nc.vector.tensor_scalar_add(dst, ps, bias_ap)
                            else:
                                nc.scalar.activation(
                                    dst, ps, mybir.ActivationFunctionType.Identity, bias=bias_ap
                                )
                            eng = nc.sync if out_q % 2 == 0 else nc.gpsimd
                            out_q += 1
                            eng.dma_start(out=o_slice, in_=dst)

    nc.compile()
    return nc


def _get_module():
    if "nc" not in _CACHE:
        _CACHE["nc"] = _build_module()
    return _CACHE["nc"]


def kernel(x, weight, bias):
    x = np.asarray(x, dtype=np.float32)
    weight = np.asarray(weight, dtype=np.float32)
    bias = np.asarray(bias, dtype=np.float32)

    xp = np.pad(x, ((0, 0), (0, 0), (1, 1), (1, 1))).reshape(32, C_IN, SP)
    xh = xp.astype(NP8)
    xl = (xp - xh.astype(np.float32)).astype(NP8)

    # weight (O, I, 3, 3) -> [I, tap, O] fp8 hi + lo
    wt = np.ascontiguousarray(weight.transpose(1, 2, 3, 0)).reshape(C_IN, NTAP, C_OUT)
    wh = wt.astype(NP8)
    wlv = (wt - wh.astype(np.float32)).astype(NP8)
    wh_s = wh.reshape(C_IN, NTAP, 2, 128).transpose(0, 2, 1, 3)  # [c, o2, tap, 128]
    wl_s = wlv.reshape(C_IN, NTAP, 2, 128).transpose(0, 2, 1, 3)
    # wts: [c, o2, slot24, 128]
    wts = np.empty((C_IN, 2, 24, 128), dtype=NP8)
    for t in range(NTAP):
        wts[:, :, 2 * t] = wh_s[:, :, t]
        wts[:, :, 2 * t + 1] = wh_s[:, :, t]
    for p in range(3):
        wts[:, :, 18 + 2 * p] = wl_s[:, :, p]
        wts[:, :, 18 + 2 * p + 1] = wl_s[:, :, p + 3]
    wts = np.ascontiguousarray(wts).reshape(C_IN, -1)
    br = np.ascontiguousarray(bias.reshape(2, 128).T)

    nc = _get_module()
    in_maps = [
        {
            "x8": np.ascontiguousarray(
                np.stack([xh[N_IMG * c : N_IMG * (c + 1)], xl[N_IMG * c : N_IMG * (c + 1)]])
            ),
            "wts": wts,
            "br": br,
        }
        for c in range(N_CORES)
    ]
    res = run_bass_kernel_spmd(nc, in_maps, core_ids=list(range(N_CORES)))
    outs = [
        np.asarray(r["out"]).astype(np.float32).reshape(N_IMG, C_OUT, H, W)
        for r in res.results
    ]
    return np.concatenate(outs, axis=0)



# revision 8
# speedup vs baseline: 1.0015x; 1.0015x over previous
"""Conv2d(128->256, 3x3, stride 1, pad 1) on (32,128,56,56) fp32, data-parallel
over 8 NeuronCores, via 1D Winograd F(4,3) along W + direct 3-tap accumulation
along H, computed in fp8e4 (e4m3) DoubleRow matmuls.

Host precomputes the W-direction input transform V = B^T d (6 taps m per 4
output cols) in f32 and ships it as fp8 hi+lo planes; weights get the G
transform (W~[kh,m] = G w[kh,:]) also split hi/lo.  Per output-row r and tap m
the device accumulates in PSUM:

    M[m] = sum_kh  (Vhi+Vlo)[r+kh] * Whi[kh,m]     (3 hi/lo DR pairs)
         +         (Vhi[r+0],Vhi[r+1]) * (Wlo[0,m],Wlo[1,m])   (1 kh DR pair)
         +         (Vhi+Vlo)[r+2] * Wlo[2,m]       (1 hi/lo DR pair)

i.e. 5 DoubleRow matmuls per M-plane, 30 per 392-column chunk (28 out rows x
14 W-tiles), 16 chunks per core (4 imgs x 2 oc-halves x 2 row-chunks).
Cost model: matmul time ~ out-cols only, so Winograd's 6 planes / 4 out-cols
beats the direct 24-ktile scheme 3.75 : 6.

The A^T output transform (Y0..Y3 from M0..M5 + bias) runs on Act/DVE/Pool
as 14 elementwise ops per chunk with bf16 intermediates (rel err ~9.2e-3).
Output leaves v-plane-major; host re-interleaves.
"""

import numpy as np
import ml_dtypes

import bass_rust
import concourse.bass as bass  # noqa: F401
import concourse.mybir as mybir
import concourse.tile as tile
from concourse import bacc
from concourse.bass_utils import run_bass_kernel_spmd

N_CORES = 8
N_IMG = 4  # images per core
C_IN = 128
C_OUT = 256
H = W = 56
RV = 58  # padded rows
T14 = 14  # W tiles (4 outputs each)
M6 = 6  # Winograd taps
ROWP = M6 * T14  # 84, V row pitch (elements)
DHL = RV * ROWP  # 4872, hi->lo plane offset
IMGP = 2 * DHL  # 9744, per-image V bytes per partition
NROW = 28  # output rows per chunk
NCH = NROW * T14  # 392 columns per PSUM plane
NSLOT = 10

F8 = mybir.dt.float8e4
NP8 = ml_dtypes.float8_e4m3
BF16 = ml_dtypes.bfloat16

ALU = mybir.AluOpType
IDENT = mybir.ActivationFunctionType.Identity

# F(4,3) transforms (Cook-Toom points 0, 1, -1, 2, -2, inf)
BT_MAT = np.array([
    [4, 0, -5, 0, 1, 0],
    [0, -4, -4, 1, 1, 0],
    [0, 4, -4, -1, 1, 0],
    [0, -2, -1, 2, 1, 0],
    [0, 2, -1, -2, 1, 0],
    [0, 4, 0, -5, 0, 1],
], dtype=np.float64)
G_MAT = np.array([
    [1 / 4, 0, 0],
    [-1 / 6, -1 / 6, -1 / 6],
    [-1 / 6, 1 / 6, -1 / 6],
    [1 / 24, 1 / 12, 1 / 6],
    [1 / 24, -1 / 12, 1 / 6],
    [0, 0, 1],
], dtype=np.float64)

_CACHE = {}


def _sv(ap_obj, dims, extra=0):
    """Hand-built (possibly overlapping) strided view of an AP."""
    c = ap_obj.copy()
    c.ap = bass_rust.VecI64Pair([list(d) for d in dims])
    c.offset = c.offset + extra
    return c


def _build_module():
    nc = bacc.Bacc("TRN2", target_bir_lowering=False, debug=False)

    f32 = mybir.dt.float32
    bf16 = mybir.dt.bfloat16
    DR = mybir.MatmulPerfMode.DoubleRow

    # v8: [img, c, hilo, row, m, t] fp8
    v8 = nc.dram_tensor("v8", [N_IMG, C_IN, IMGP], F8, kind="ExternalInput").ap()
    # wts: [c, o2, m, slot10, 128oc] fp8
    wts = nc.dram_tensor("wts", [C_IN, 2 * M6 * NSLOT * 128], F8, kind="ExternalInput").ap()
    br = nc.dram_tensor("br", [C_IN, 2], f32, kind="ExternalInput").ap()
    # out8: [img, o2, rchunk, 128, v, NCH] bf16, v-plane-major
    out = nc.dram_tensor("out", [N_IMG, 2, 2, C_IN, 4, NCH], bf16, kind="ExternalOutput").ap()

    wts_v = wts.rearrange("c (h m s o) -> c h m s o", h=2, m=M6, s=NSLOT)

    with tile.TileContext(nc) as tc:
        with (
            tc.tile_pool(name="const", bufs=1) as cpool,
            tc.tile_pool(name="tp", bufs=3) as tpool,
            tc.tile_pool(name="osb", bufs=3) as opool,
            tc.tile_pool(name="pp", bufs=8, space="PSUM") as ppool,
        ):
            v_sb = cpool.tile([C_IN, N_IMG, 2, RV, ROWP], F8, name="v_sb")
            w_sb = cpool.tile([C_IN, 2, M6, NSLOT, 128], F8, name="w_sb")
            b_sb = cpool.tile([C_IN, 2], f32, name="b_sb")

            # ---- PE clock warmup: pin pe_busy_start early; by the time the
            # head DMAs land (~3.5us) the p-state ramp has hit full clock.
            WARM_N = 64
            warm_sb = cpool.tile([C_IN, WARM_N], f32, name="warm_sb")
            nc.gpsimd.memzero(warm_sb)
            ps_warm = ppool.tile([128, NCH], f32, tag="m", name="ps_warm")
            N_WARM = 8
            for i in range(N_WARM):
                nc.tensor.matmul(
                    ps_warm[:WARM_N, :WARM_N],
                    lhsT=warm_sb[:, :WARM_N],
                    rhs=warm_sb,
                    start=(i == 0),
                    stop=(i == N_WARM - 1),
                )

            # ---- loads (vector HWDGE queue, in issue order) ----
            nc.scalar.dma_start(out=b_sb, in_=br)
            nc.scalar.dma_start(out=w_sb[:, 0], in_=wts_v[:, 0])
            BROW = 32
            v8v = [v8[n].rearrange("c (l r w) -> c l r w", l=2, r=RV) for n in range(N_IMG)]
            nc.scalar.dma_start(out=v_sb[:, 0, :, :BROW, :], in_=v8v[0][:, :, :BROW, :])
            nc.scalar.dma_start(out=v_sb[:, 0, :, BROW:, :], in_=v8v[0][:, :, BROW:, :])
            nc.scalar.dma_start(out=w_sb[:, 1], in_=wts_v[:, 1])
            for n in range(1, N_IMG):
                nc.scalar.dma_start(out=v_sb[:, n], in_=v8v[n])

            hi0 = v_sb[:, 0, 0, :, :]
            pstride = hi0.ap[0][0]

            def chunk_matmuls(ps, n, o2, m, r0):
                base = v_sb[:, n, 0, :, :]
                ndims = [[ROWP, NROW], [1, T14]]

                def rhs(pair_d, kh):
                    return _sv(
                        base,
                        [[pstride, 128], [pair_d, 2]] + ndims,
                        extra=(r0 + kh) * ROWP + m * T14,
                    )

                # 3 hi/lo pairs vs Whi[kh]
                for kh in range(3):
                    nc.tensor.matmul(
                        ps, lhsT=w_sb[:, o2, m, 2 * kh: 2 * kh + 2], rhs=rhs(DHL, kh),
                        start=(kh == 0), stop=False, perf_mode=DR,
                    )
                # kh-pair (0,1) vs (Wlo0, Wlo1)
                nc.tensor.matmul(
                    ps, lhsT=w_sb[:, o2, m, 6:8], rhs=rhs(ROWP, 0),
                    start=False, stop=False, perf_mode=DR,
                )
                # hi/lo pair at kh=2 vs Wlo2
                nc.tensor.matmul(
                    ps, lhsT=w_sb[:, o2, m, 8:10], rhs=rhs(DHL, 2),
                    start=False, stop=True, perf_mode=DR,
                )

            for n in range(N_IMG):
                for o2 in range(2):
                    bias_ap = b_sb[:, o2: o2 + 1]
                    for rc in range(2):
                        r0 = rc * NROW
                        ps = {}
                        for m in (2, 4, 3, 1, 5, 0):
                            ps[m] = ppool.tile([128, NCH], f32, tag="m", name=f"m{m}")
                            chunk_matmuls(ps[m], n, o2, m, r0)

                        t = {
                            name: tpool.tile([128, NCH], f32 if name == "d34" else bf16,
                                             tag=name, name=name)
                            for name in ("c2b", "c2nb", "c4", "c5", "c0", "s34", "d34",
                                         "s12b", "d12b", "y0a", "y3a")
                        }
                        o_sb = opool.tile([128, 4, NCH], bf16, tag="o", name="o_sb")

                        # Act: PSUM->SBUF copies (+/- bias).  GPSIMD cannot
                        # read PSUM (and cannot run TensorScalarPtr), so
                        # Act+DVE do all plane exits and scalar ops; Pool gets
                        # plain SBUF adds only.
                        nc.scalar.activation(t["c2b"], ps[2], IDENT, bias=bias_ap)
                        nc.scalar.activation(t["c2nb"], ps[2], IDENT, bias=bias_ap, scale=-1.0)
                        nc.scalar.activation(t["c4"], ps[4], IDENT)
                        nc.scalar.activation(t["c5"], ps[5], IDENT)
                        nc.scalar.activation(t["c0"], ps[0], IDENT)
                        # DVE: PSUM-fused combines + scalar-coefficient ops
                        nc.vector.tensor_tensor(t["s34"], ps[3], t["c4"], ALU.add)
                        nc.vector.scalar_tensor_tensor(t["d34"], t["c4"], -1.0, ps[3],
                                                       ALU.mult, ALU.add)
                        nc.vector.tensor_tensor(t["s12b"], ps[1], t["c2b"], ALU.add)
                        nc.vector.tensor_tensor(t["d12b"], ps[1], t["c2nb"], ALU.add)
                        nc.vector.scalar_tensor_tensor(t["y3a"], t["d34"], 8.0, t["c5"],
                                                       ALU.mult, ALU.add)
                        nc.vector.scalar_tensor_tensor(o_sb[:, 1, :], t["d34"], 2.0,
                                                       t["d12b"], ALU.mult, ALU.add)
                        nc.vector.scalar_tensor_tensor(o_sb[:, 2, :], t["s34"], 4.0,
                                                       t["s12b"], ALU.mult, ALU.add)
                        # Pool: plain bf16 SBUF adds
                        nc.gpsimd.tensor_tensor(t["y0a"], t["c0"], t["s34"], ALU.add)
                        nc.gpsimd.tensor_tensor(o_sb[:, 0, :], t["y0a"], t["s12b"], ALU.add)
                        nc.gpsimd.tensor_tensor(o_sb[:, 3, :], t["y3a"], t["d12b"], ALU.add)

                        nc.sync.dma_start(out=out[n, o2, rc], in_=o_sb)

    nc.compile()
    return nc


def _get_module():
    if "nc" not in _CACHE:
        _CACHE["nc"] = _build_module()
    return _CACHE["nc"]


def kernel(x, weight, bias):
    x = np.asarray(x, dtype=np.float32)
    weight = np.asarray(weight, dtype=np.float32)
    bias = np.asarray(bias, dtype=np.float32)

    NB = x.shape[0]
    xp = np.pad(x, ((0, 0), (0, 0), (1, 1), (1, 1)))  # (32,128,58,58)
    s = xp.strides
    tiles = np.lib.stride_tricks.as_strided(
        xp, shape=(NB, C_IN, RV, T14, 6), strides=(s[0], s[1], s[2], 4 * s[3], s[3])
    )
    # V[n,c,row,t,m] -> [n,c,row,m,t]
    V = np.matmul(tiles, BT_MAT.T.astype(np.float32)).swapaxes(3, 4)
    Vhi = V.astype(NP8)
    Vlo = (V - Vhi.astype(np.float32)).astype(NP8)
    # [n, c, hilo, row, m, t]
    v8 = np.ascontiguousarray(
        np.stack([Vhi, Vlo], axis=2).reshape(NB, C_IN, IMGP)
    )

    # W~[kh, m, O, I] = sum_kw G[m,kw] w[O,I,kh,kw]
    Wt = np.einsum("mw,oihw->hmoi", G_MAT, weight.astype(np.float64)).astype(np.float32)
    Whi = Wt.astype(NP8)
    Wlo = (Wt - Whi.astype(np.float32)).astype(NP8)

    def iomo(a):  # [m,O,I] -> [I, m, o2, 128]
        return a.transpose(2, 0, 1).reshape(C_IN, M6, 2, 128)

    wts = np.empty((C_IN, 2, M6, NSLOT, 128), dtype=NP8)
    hi = np.stack([iomo(Whi[kh]) for kh in range(3)])  # [kh, I, m, o2, 128]
    lo = np.stack([iomo(Wlo[kh]) for kh in range(3)])
    for kh in range(3):
        wts[:, :, :, 2 * kh] = hi[kh].transpose(0, 2, 1, 3)
        wts[:, :, :, 2 * kh + 1] = hi[kh].transpose(0, 2, 1, 3)
    wts[:, :, :, 6] = lo[0].transpose(0, 2, 1, 3)
    wts[:, :, :, 7] = lo[1].transpose(0, 2, 1, 3)
    wts[:, :, :, 8] = lo[2].transpose(0, 2, 1, 3)
    wts[:, :, :, 9] = lo[2].transpose(0, 2, 1, 3)
    wts = np.ascontiguousarray(wts).reshape(C_IN, -1)

    br = np.ascontiguousarray(bias.reshape(2, 128).T)

    nc = _get_module()
    in_maps = [
        {"v8": v8[N_IMG * c: N_IMG * (c + 1)], "wts": wts, "br": br}
        for c in range(N_CORES)
    ]
    res = run_bass_kernel_spmd(nc, in_maps, core_ids=list(range(N_CORES)))
    outs = []
    for r in res.results:
        o = np.asarray(r["out"]).astype(np.float32)
        o = o.reshape(N_IMG, 2, 2, 128, 4, NROW, T14)
        # [n, o2, rc, oc, v, r, t] -> [n, o2, oc, rc, r, t, v]
        o = o.transpose(0, 1, 3, 2, 5, 6, 4).reshape(N_IMG, C_OUT, H, W)
        outs.append(o)
    return np.concatenate(outs, axis=0)
